# revision 9
# baseline (speedup 1.0000x reference)
"""Trainium2 Bass kernel for ConvPool (3x3 VALID conv + bias + relu + 2x2 maxpool).

Full-input contract: kernel(x, weight, bias) -> (32, 64, 3969) float32.
Data-parallel over batch across 8 NeuronCores (4 images per core).

Per-core algorithm (v2 — PE-tiled concurrent matmuls):
  - The 128x128 PE array is addressed as two independent 64x64 tiles
    (row bands 0-63 / 64-127).  Two images run concurrently, one per band:
    image chains are staggered one conv-tile apart so the two bands never
    touch the same PSUM bank simultaneously (a HW restriction).
  - Per image, 4 physical copies of x live in its 64-partition band
    (shifts 1, 2, 0, W at sub-bases +0/+16/+32/+48; the shift-0 master is
    DMA'd from HBM with a bf16 cast, the other three are SBUF->SBUF DMAs
    off the master).  The 9 conv taps are covered by 3 accumulating
    matmuls (K=64,48,48) whose free-dim offsets select the vertical tap:
      MM1 off r*W:     taps (0,1) (0,2) (0,0) (1,0)
      MM2 off (r+1)*W: taps (1,1) (1,2) [zero] (x)    K=48
      MM3 off (r+2)*W: taps (2,1) (2,2) (2,0)         K=48
    (MM2's third 16-row block duplicates tap (1,0), so its weights are 0.)
  - Post per PSUM bank [128, 4 rows x 128 cols]: DVE h-max (column pairs,
    PSUM->SBUF), Pool v-max (row pairs), ACT relu(x+bias) into the output
    stage; one 2 MB DMA per image pair.  relu+bias commute with max-pool.
"""

import contextlib

import numpy as np

import concourse.bass as bass
import concourse.bacc as bacc
import concourse.mybir as mybir
import concourse.tile as tile
from concourse.bass_utils import run_bass_kernel_spmd

N_CORES = 8
B, C, H, W = 32, 16, 128, 128
FD, OUT, POOL = 3, 64, 2
BPC = B // N_CORES            # images per core
HC = H - FD + 1               # conv output height/width = 126
HP = HC // POOL               # pooled height/width = 63
NPIX = HP * HP                # 3969
RPT = 4                       # conv rows per PSUM tile (N = 4*128 = 512)
HW = H * W

f32 = mybir.dt.float32
bf16 = mybir.dt.bfloat16
MAX = mybir.AluOpType.max

_cache: dict = {}

# Sub-band partition bases for the four x copies (shift -> base offset).
CP1, CP2, CP0, CP3 = 0, 16, 32, 48     # shifts 1, 2, 0, W


def _load_weights(nc, w_sb, w_d):
    """w_sb[128, 3*OUT]: lhsT blocks for MM1/MM2/MM3, per 64-row band.

    Within a band, 16-row groups hold (in order) the x copies with shifts
    1, 2, 0, W.  Block k (cols k*OUT:(k+1)*OUT) is the stationary operand
    of MMk+1; the (n, m) tap index of each group follows from shift + off.
    """
    w_view = w_d.rearrange("(c n m) o -> n m c o", c=C, n=FD, m=FD)
    # (copy_base, mm_block, n, m); MM2's copy0 rows stay zero (memset).
    slots = [
        (CP1, 0, 0, 1), (CP2, 0, 0, 2), (CP0, 0, 0, 0), (CP3, 0, 1, 0),
        (CP1, 1, 1, 1), (CP2, 1, 1, 2),
        (CP1, 2, 2, 1), (CP2, 2, 2, 2), (CP0, 2, 2, 0),
    ]
    for band in (0, 64):
        for cp, blk, n, m in slots:
            p0 = band + cp
            nc.gpsimd.dma_start(
                w_sb[p0:p0 + C, blk * OUT:(blk + 1) * OUT], w_view[n, m])


def _load_image(nc, x_rep, band, x_flat):
    """HBM master + 3 SBUF->SBUF shifted replicas for one image's band."""
    master = x_rep[band + CP0:band + CP0 + C, :]
    nc.gpsimd.dma_start(master, x_flat)      # fp32 -> bf16 cast: SWDGE only
    nc.sync.dma_start(
        x_rep[band + CP1:band + CP1 + C, 0:HW - 1], master[:, 1:])
    nc.sync.dma_start(
        x_rep[band + CP2:band + CP2 + C, 0:HW - 2], master[:, 2:])
    nc.scalar.dma_start(
        x_rep[band + CP3:band + CP3 + C, 0:HW - W], master[:, W:])


def _post(nc, ps, ostage, bias_sb, rpool, hpool, t):
    """ACT evac relu(ps+bias) -> DVE h-max -> Pool v-max into ostage.

    (tensor_tensor cannot take two PSUM operands — PSUM has one DVE read
    port — so the PSUM evacuation must be the single-input ACT op.)
    """
    r0 = RPT * t
    nrows = min(RPT, HC - r0)
    npr = nrows // 2
    ncols = nrows * W
    rb = rpool.tile([128, RPT * W], f32)
    nc.scalar.activation(
        rb[:, 0:ncols], ps[:, 0:ncols],
        mybir.ActivationFunctionType.Relu,
        bias=bias_sb[:, 0:1], scale=1.0)
    hb = hpool.tile([128, RPT * HP], f32)
    rb_v = rb.rearrange("p (r j two) -> p r j two", two=2, j=W // 2)
    hb_v = hb.rearrange("p (r j) -> p r j", j=HP)
    nc.vector.tensor_tensor(
        out=hb_v[:, 0:nrows, :],
        in0=rb_v[:, 0:nrows, 0:HP, 0],
        in1=rb_v[:, 0:nrows, 0:HP, 1],
        op=MAX)
    q0 = 2 * t * HP
    ov = ostage[:, q0:q0 + npr * HP].rearrange("p (pr j) -> p pr j", j=HP)
    hb_p = hb.rearrange("p (pr two j) -> p pr two j", two=2, j=HP)
    nc.vector.tensor_tensor(
        out=ov,
        in0=hb_p[:, 0:npr, 0, :],
        in1=hb_p[:, 0:npr, 1, :],
        op=MAX)


def _build(loop_reps: int | None = None, mode: str = "full"):
    """Build the per-core program.  loop_reps wraps the body in a hardware
    For_i loop (benchmarking).  mode: full | nopost | dmaonly | mmonly."""
    nc = bacc.Bacc("TRN2", target_bir_lowering=False, debug=False)
    x_d = nc.dram_tensor("x", [BPC, C, H, W], f32, kind="ExternalInput").ap()
    w_d = nc.dram_tensor("weight", [C * FD * FD, OUT], f32,
                         kind="ExternalInput").ap()
    b_d = nc.dram_tensor("bias", [OUT], f32, kind="ExternalInput").ap()
    y_d = nc.dram_tensor("y", [BPC, OUT, NPIX], f32, kind="ExternalOutput").ap()

    n_tiles = (HC + RPT - 1) // RPT      # 32

    with tile.TileContext(nc) as tc:
        with (
            tc.tile_pool(name="const", bufs=1) as const,
            tc.tile_pool(name="xrep", bufs=3) as xpool,
            tc.tile_pool(name="psum", bufs=6, space="PSUM") as psum,
            tc.tile_pool(name="rbuf", bufs=3) as rpool,
            tc.tile_pool(name="hbuf", bufs=3) as hpool,
            tc.tile_pool(name="ostage", bufs=2) as opool,
        ):
            w_sb = const.tile([128, FD * OUT], bf16)
            nc.vector.memset(w_sb[:], 0.0)
            _load_weights(nc, w_sb, w_d)

            bias_sb = const.tile([128, 1], f32)
            b_src = b_d.rearrange("(o u) -> o u", u=1)
            nc.sync.dma_start(bias_sb[0:OUT, :], b_src)
            nc.sync.dma_start(bias_sb[OUT:128, :], b_src)

            if mode == "mmonly":
                x_reps = []
                for p in range(BPC // 2):
                    x_rep = const.tile([128, HW], bf16)
                    for img in range(2):
                        x_flat = x_d[2 * p + img].rearrange("c h w -> c (h w)")
                        _load_image(nc, x_rep, img * 64, x_flat)
                    x_reps.append(x_rep)

            loop_cm = (tc.For_i(0, loop_reps, 1) if loop_reps
                       else contextlib.nullcontext())
            with loop_cm:
                for p in range(BPC // 2):
                    if mode == "mmonly":
                        x_rep = x_reps[p]
                    else:
                        x_rep = xpool.tile([128, HW], bf16)
                        for img in range(2):
                            x_flat = x_d[2 * p + img].rearrange(
                                "c h w -> c (h w)")
                            _load_image(nc, x_rep, img * 64, x_flat)
                    if mode == "dmaonly":
                        continue

                    ostage = opool.tile([128, NPIX], f32)
                    # Staggered dual-band chains: step s runs img0 tile s
                    # concurrently with img1 tile s-1 (different PSUM banks).
                    ps_t = {}
                    for s in range(n_tiles + 1):
                        mm = []
                        if s < n_tiles:
                            ps_t[s] = psum.tile([128, RPT * W], f32,
                                                name="ps")
                            mm.append((0, s))
                        if s >= 1:
                            mm.append((1, s - 1))
                        # interleave the two chains matmul-by-matmul
                        for k in range(FD):
                            for img, t in mm:
                                band = img * 64
                                r0 = RPT * t
                                nrows = min(RPT, HC - r0)
                                ncols = nrows * W
                                koff, kk = ((0, 64), (1, 48), (2, 48))[k]
                                off = (r0 + koff) * W
                                nc.tensor.matmul(
                                    ps_t[t][band:band + OUT, 0:ncols],
                                    w_sb[band:band + kk,
                                         k * OUT:(k + 1) * OUT],
                                    x_rep[band:band + kk, off:off + ncols],
                                    start=(k == 0), stop=(k == 2))
                        if s >= 1 and mode == "full":
                            _post(nc, ps_t[s - 1], ostage, bias_sb,
                                  rpool, hpool, s - 1)

                    if mode == "full":
                        dst = y_d[2 * p:2 * p + 2].rearrange("b o q -> (b o) q")
                        nc.scalar.dma_start(dst, ostage[:])

    nc.compile()
    return nc


def _get_nc():
    if "nc" not in _cache:
        _cache["nc"] = _build()
    return _cache["nc"]


def kernel(x: np.ndarray, weight: np.ndarray, bias: np.ndarray) -> np.ndarray:
    nc = _get_nc()
    x = np.ascontiguousarray(x, dtype=np.float32)
    weight = np.ascontiguousarray(weight, dtype=np.float32)
    bias = np.ascontiguousarray(bias, dtype=np.float32)
    xs = x.reshape(N_CORES, BPC, C, H, W)
    in_maps = [{"x": xs[i], "weight": weight, "bias": bias}
               for i in range(N_CORES)]
    res = run_bass_kernel_spmd(nc, in_maps, list(range(N_CORES)))
    return np.concatenate([res.results[i]["y"] for i in range(N_CORES)], axis=0)


# revision 20
# speedup vs baseline: 2.0255x; 2.0255x over previous
"""Trainium2 Bass kernel for ConvPool (3x3 VALID conv + bias + relu + 2x2 maxpool).

Full-input contract: kernel(x, weight, bias) -> (32, 64, 3969) float32.
Data-parallel over batch across 8 NeuronCores (4 images per core).

Per-core algorithm (v3 — PE-tiled concurrent matmuls, pooled-from-PSUM):
  - The 128x128 PE array is addressed as two independent 64x64 tiles
    (row bands 0-63 / 64-127).  Two images run concurrently, one per band;
    their 3-matmul accumulation chains are staggered one conv-row-tile
    apart and interleaved per-matmul, so at any instant the two bands
    stream into DIFFERENT PSUM banks (same-bank concurrent access from
    different row tiles is a HW restriction).
  - Per image, 4 physical copies of x live in its 64-partition band
    (shifts 1, 2, 0, W at sub-bases +0/+16/+32/+48; the shift-0 master is
    DMA'd from HBM with a bf16 cast, the other three are SBUF->SBUF DMAs
    off the master).  The 9 conv taps are covered by 3 accumulating
    matmuls (K=64,48,48) whose free-dim offsets select the vertical tap:
      MM1 off r*W:     taps (0,1) (0,2) (0,0) (1,0)      K=64
      MM2 off (r+1)*W: taps (1,1) (1,2) [zero w]         K=48
      MM3 off (r+2)*W: taps (2,1) (2,2) (2,0)            K=48
    (MM2's 16 master rows duplicate tap (1,0), so their weights are 0.)
  - The matmul out AP is 2x2-BLOCKED: conv pixel (r, c) lands at PSUM
    address (r//2)*256 + (c//2)*4 + (r%2)*2 + (c%2), so each 2x2 pooling
    window is contiguous.  One DVE pool_max per bank then produces the
    pooled row pair straight from PSUM (the only single-input op that can
    max-reduce; tensor_tensor cannot take two PSUM operands).
  - relu(x + bias) is applied once per image pair on the pooled stage
    (64x fewer ACT instructions than evacuating conv tiles; max-pool
    commutes with the monotone x -> relu(x+b)), then one 2 MB DMA out.
"""

import contextlib

import numpy as np

import concourse.bass as bass
import concourse.bacc as bacc
import concourse.mybir as mybir
import concourse.tile as tile
from concourse.bass_utils import run_bass_kernel_spmd

N_CORES = 8
B, C, H, W = 32, 16, 128, 128
FD, OUT, POOL = 3, 64, 2
BPC = B // N_CORES            # images per core
HC = H - FD + 1               # conv output height/width = 126
HP = HC // POOL               # pooled height/width = 63
NPIX = HP * HP                # 3969
RPT = 4                       # conv rows per PSUM tile (N = 4*128 = 512)
HW = H * W

f32 = mybir.dt.float32
bf16 = mybir.dt.bfloat16

_cache: dict = {}

# Sub-band partition bases for the four x copies (shift -> base offset).
CP1, CP2, CP0, CP3 = 0, 16, 32, 48     # shifts 1, 2, 0, W


def _load_weights(nc, w_sb, w_d):
    """w_sb[128, 3*OUT]: lhsT blocks for MM1/MM2/MM3, per 64-row band."""
    w_view = w_d.rearrange("(c n m) o -> n m c o", c=C, n=FD, m=FD)
    # (copy_base, mm_block, n, m); MM2's master rows stay zero (memset).
    slots = [
        (CP1, 0, 0, 1), (CP2, 0, 0, 2), (CP0, 0, 0, 0), (CP3, 0, 1, 0),
        (CP1, 1, 1, 1), (CP2, 1, 1, 2),
        (CP1, 2, 2, 1), (CP2, 2, 2, 2), (CP0, 2, 2, 0),
    ]
    for band in (0, 64):
        for cp, blk, n, m in slots:
            p0 = band + cp
            nc.gpsimd.dma_start(
                w_sb[p0:p0 + C, blk * OUT:(blk + 1) * OUT], w_view[n, m])


def _load_image(nc, x_rep, band, x_flat):
    """HBM master + 3 SBUF->SBUF shifted replicas for one image's band."""
    master = x_rep[band + CP0:band + CP0 + C, :]
    nc.gpsimd.dma_start(master, x_flat)      # fp32 -> bf16 cast: SWDGE only
    nc.sync.dma_start(
        x_rep[band + CP1:band + CP1 + C, 0:HW - 1], master[:, 1:])
    nc.sync.dma_start(
        x_rep[band + CP2:band + CP2 + C, 0:HW - 2], master[:, 2:])
    nc.scalar.dma_start(
        x_rep[band + CP3:band + CP3 + C, 0:HW - W], master[:, W:])


def _mm(nc, w_sb, x_rep, ps, band, t, k, r2):
    """Matmul (k, r2) of the 6-matmul chain for conv-row tile t on one band.

    Rows of parity r2 only (matmul APs are limited to 3 free dims); the out
    AP is 2x2-blocked so each pooling window is contiguous in PSUM:
    conv pixel (r, c) -> address (r//2)*256 + (c//2)*4 + (r%2)*2 + (c%2).
    """
    r0 = RPT * t
    nrows = min(RPT, HC - r0)
    nrr = nrows // 2
    koff, kk = ((0, 64), (1, 48), (2, 48))[k]
    rbase = r0 + koff + r2
    xv = x_rep[band:band + kk, :].rearrange("q (r w) -> q r w", w=W)
    rhs = xv[:, rbase:rbase + 2 * nrr - 1:2, :].rearrange(
        "q rr (j par) -> q rr j par", j=W // 2, par=2)
    out = ps[band:band + OUT, :].rearrange(
        "p (rr j f) -> p rr j f", rr=2, j=W // 2, f=4)[
        :, 0:nrr, :, 2 * r2:2 * r2 + 2]
    # start=True zeroes the addressed partitions' whole bank row, so only
    # the chain's first matmul may carry it; later writes to untouched
    # elements store via the cleared has_written bits.
    nc.tensor.matmul(out, w_sb[band:band + kk, k * OUT:(k + 1) * OUT], rhs,
                     start=(k == 0 and r2 == 0), stop=(k == 2 and r2 == 1))


def _pool(nc, ps, ostage, t):
    """One DVE pool_max over each contiguous 2x2 block, PSUM -> ostage."""
    nrows = min(RPT, HC - RPT * t)
    nrr = nrows // 2
    pv = ps[:, 0:nrows * W].rearrange(
        "p (rr j w) -> p rr j w", rr=nrr, j=W // 2, w=4)
    q0 = 2 * t * HP
    ov = ostage[:, q0:q0 + nrr * HP].rearrange("p (rr j) -> p rr j", j=HP)
    nc.vector.tensor_reduce(out=ov, in_=pv[:, :, 0:HP, :],
                            axis=mybir.AxisListType.X,
                            op=mybir.AluOpType.max)


def _build(loop_reps: int | None = None, mode: str = "full"):
    """Build the per-core program.  loop_reps wraps the body in a hardware
    For_i loop (benchmarking).  mode: full | pool | nopost | dmaonly | mmonly.
    """
    nc = bacc.Bacc("TRN2", target_bir_lowering=False, debug=False)
    x_d = nc.dram_tensor("x", [BPC, C, H, W], f32, kind="ExternalInput").ap()
    w_d = nc.dram_tensor("weight", [C * FD * FD, OUT], f32,
                         kind="ExternalInput").ap()
    b_d = nc.dram_tensor("bias", [OUT], f32, kind="ExternalInput").ap()
    y_d = nc.dram_tensor("y", [BPC, OUT, NPIX], f32, kind="ExternalOutput").ap()

    n_tiles = (HC + RPT - 1) // RPT      # 32

    with tile.TileContext(nc) as tc:
        with (
            tc.tile_pool(name="const", bufs=1) as const,
            tc.tile_pool(name="xrep", bufs=3) as xpool,
            tc.tile_pool(name="psum", bufs=6, space="PSUM") as psum,
            tc.tile_pool(name="ostage", bufs=2) as opool,
            tc.tile_pool(name="ofin", bufs=2) as fpool,
        ):
            w_sb = const.tile([128, FD * OUT], bf16)
            nc.vector.memset(w_sb[:], 0.0)
            _load_weights(nc, w_sb, w_d)

            bias_sb = const.tile([128, 1], f32)
            b_src = b_d.rearrange("(o u) -> o u", u=1)
            nc.sync.dma_start(bias_sb[0:OUT, :], b_src)
            nc.sync.dma_start(bias_sb[OUT:128, :], b_src)

            loop_cm = (tc.For_i(0, loop_reps, 1) if loop_reps
                       else contextlib.nullcontext())
            with loop_cm:
                for p in range(BPC // 2):
                    x_rep = xpool.tile([128, HW], bf16)
                    if mode == "mmonly":
                        # token load only: keep the dep structure, no DMA cost
                        nc.gpsimd.dma_start(
                            x_rep[0:C, 0:512],
                            x_d[2 * p].rearrange("c h w -> c (h w)")[:, 0:512])
                    else:
                        for img in range(2):
                            x_flat = x_d[2 * p + img].rearrange(
                                "c h w -> c (h w)")
                            _load_image(nc, x_rep, img * 64, x_flat)
                    if mode == "dmaonly":
                        continue

                    ostage = opool.tile([128, NPIX], f32)
                    # Staggered dual-band chains: step s runs img0 tile s
                    # concurrently with img1 tile s-1 (different PSUM banks).
                    ps_t = {}
                    for s in range(n_tiles + 1):
                        jobs = []
                        if s < n_tiles:
                            ps_t[s] = psum.tile([128, RPT * W], f32,
                                                name="ps")
                            jobs.append((0, s))
                        if s >= 1:
                            jobs.append((64, s - 1))
                        for k in range(FD):
                            for r2 in range(2):
                                for band, t in jobs:
                                    _mm(nc, w_sb, x_rep, ps_t[t], band, t,
                                        k, r2)
                        if s >= 1 and mode in ("full", "pool"):
                            _pool(nc, ps_t[s - 1], ostage, s - 1)

                    if mode == "full":
                        ofin = fpool.tile([128, NPIX], f32)
                        nc.scalar.activation(
                            ofin[:], ostage[:],
                            mybir.ActivationFunctionType.Relu,
                            bias=bias_sb[:, 0:1], scale=1.0)
                        dst = y_d[2 * p:2 * p + 2].rearrange("b o q -> (b o) q")
                        nc.scalar.dma_start(dst, ofin[:])

    nc.compile()
    return nc


def _get_nc():
    if "nc" not in _cache:
        _cache["nc"] = _build()
    return _cache["nc"]


def kernel(x: np.ndarray, weight: np.ndarray, bias: np.ndarray) -> np.ndarray:
    nc = _get_nc()
    x = np.ascontiguousarray(x, dtype=np.float32)
    weight = np.ascontiguousarray(weight, dtype=np.float32)
    bias = np.ascontiguousarray(bias, dtype=np.float32)
    xs = x.reshape(N_CORES, BPC, C, H, W)
    in_maps = [{"x": xs[i], "weight": weight, "bias": bias}
               for i in range(N_CORES)]
    res = run_bass_kernel_spmd(nc, in_maps, list(range(N_CORES)))
    return np.concatenate([res.results[i]["y"] for i in range(N_CORES)], axis=0)
